# revision 27
# baseline (speedup 1.0000x reference)
"""Depthwise 3x3 CNN combo kernel for TRN2 (8 NeuronCores, channel-parallel).

Computes  out = relu(x*a0 + dwconv(x,w1)*a1 + dwconv(x,w2)*a2 + dwconv(x,w3)*a3)
for x [8, 256, 128, 128] f32 by folding everything into a single 9-tap
depthwise conv (conv is linear in the weights; the residual a0*x is the
center tap):  w_eff = a1*w1 + a2*w2 + a3*w3,  w_eff[:,1,1] += a0.

Sharding: CHANNELS across the 8 cores (32 channels x 8 batch images per
core).  Per-core layout puts image ROWS on the partitions:

  x tile  [y=128, (c, b, w=132)]   (w padded 2 left / 2 right with zeros,
                                    host-prepadded so DMA runs are 2112B)

The vertical 3-tap conv then becomes a matmul over the partition (row)
dim with a TRIDIAGONAL stationary matrix T_dx[yi, yo] = w_eff[c, yi-yo+1, dx]
(one matrix per channel and horizontal offset dx).  Each streamed rhs
column picks up all 3 vertical taps at once (384 useful MACs/cycle vs
128 for a diagonal matmul), so the full 9-tap conv needs only 3
accumulating matmuls per psum tile:

  psum[yo, (b, t)] += sum_yi T_dx[yi, yo] * xt[yi, (b, t + dx)]   dx = 0..2

with the dx shift handled as a free-dim offset into the padded row.
Row-boundary zero padding falls out of the band truncating at the matrix
edge.  PSUM accumulates in f32; relu+downcast to bf16 runs on ScalarE
(2/3) and DVE (1/3); host upcasts.

The tridiagonal matrices are built on DVE from 3 host-supplied one-hot
diagonal masks and per-(c,dx,diag) scalars (3 small ops per matrix), so
the only HBM traffic is x in (8.7MB) and y out (8.4MB) per core.
"""

import numpy as np

import concourse.bacc as bacc
import concourse.mybir as mybir
from concourse import bass_utils
from concourse.tile import TileContext

# Problem constants (hardcoded per contract).
B = 8
C = 256
H = 128
W = 128
NCORES = 8

CPC = C // NCORES   # channels per core
P = 128             # partitions (= H rows)
WP = W + 2          # padded row width (1 zero col each side)

F32 = mybir.dt.float32
BF16 = mybir.dt.bfloat16

# "host": DMA full tridiag matrices from HBM.  "dve": build them on-chip
# from diagonal masks + per-channel scalars.  Measured: DVE builds run at
# 1x mode (~1us/matrix, 95us total) while the 3MB host DMA costs ~9us,
# so "host" wins decisively.
W_MODE = "host"


def build_tile_kernel(tc, y_ap, x_ap, w_ap):
    nc = tc.nc
    relu = mybir.ActivationFunctionType.Relu

    with (
        tc.tile_pool(name="xpool", bufs=16) as xpool,
        tc.tile_pool(name="wpool", bufs=8) as wpool,
        tc.tile_pool(name="psum", bufs=4, space="PSUM") as psum_pool,
        tc.tile_pool(name="opool", bufs=4) as opool,
    ):
        # DMA plumbing facts (measured): only sync (SP) and scalar
        # (Activation) have HWDGE queues, ~215GB/s each, with a ~6-deep
        # trigger ring -- a 7th trigger BLOCKS the issuing engine queue.
        # The gpsimd SWDGE path adds ~200GB/s more but starts slowly.
        # Feeding the PE (1.3us/channel) needs ~485GB/s, so: outputs for
        # middle pairs ride SWDGE, x moves in 2-channel chunks and W in
        # 4-channel chunks alternating between the two HWDGE queues, and
        # triggers are emitted just-in-time from inside the channel loop
        # (~8 channels ahead) so ScalarE relus never sit behind a blocked
        # trigger and per-queue FIFO order matches need order.
        GP_PAIRS = (4, 5, 6, 7, 8, 9)  # output pairs on gpsimd SWDGE

        xts = [None] * 16  # 2-channel x chunks, c = 2k
        wts = [None] * 8   # 4-channel W chunks, c = 4j

        def emit_x(k):
            c0 = 2 * k
            xt = xpool.tile([P, 2, B, WP], BF16, name="xt", tag="xt")
            q = nc.sync if k % 2 == 0 else nc.scalar
            q.dma_start(xt[:], x_ap[:, c0 : c0 + 2])
            xts[k] = xt

        def emit_w(j):
            c0 = 4 * j
            wc = wpool.tile([P, 4, 3, P], BF16, name="wc", tag="wc")
            q = nc.scalar if j % 2 == 0 else nc.sync
            q.dma_start(wc[:], w_ap[:, c0 : c0 + 4])
            wts[j] = wc

        # Prefetch channels 0-7 (need order, <=3 triggers per queue).
        emit_x(0); emit_w(0); emit_x(1); emit_x(2); emit_w(1); emit_x(3)

        ot = None
        for c in range(CPC):
            if c % 2 == 0:
                k = c // 2 + 4
                if k < 16:
                    emit_x(k)
            if c % 4 == 0:
                j = c // 4 + 2
                if j < 8:
                    emit_w(j)
            if c % 2 == 0:
                # 2-channel output tile -> one 4KB-run DMA per 2 channels.
                ot = opool.tile([P, 2, B, W], BF16, name="ot", tag="ot")
            # One 2-bank psum tile per channel: 2 groups of 4 images, each
            # matmul exactly one full bank (N = 4*128 = 512, no garbage
            # columns thanks to the per-dx restricted rhs window).  4
            # channels in flight; relu is ONE contiguous op per channel.
            ps = psum_pool.tile([P, 2, 512], F32, name="ps", tag="ps")
            for dx in range(3):
                lhsT = wts[c // 4][:, c % 4, dx, :]
                for g in range(2):
                    nc.tensor.matmul(
                        ps[:, g, :],
                        lhsT=lhsT,
                        rhs=xts[c // 2][:, c % 2, 4 * g : 4 * g + 4,
                                        dx : dx + W],
                        start=(dx == 0),
                        stop=(dx == 2),
                        skip_group_check=True,
                    )
            # relu + bf16 downcast, alternating DVE / ScalarE.  (GpSimd
            # cannot read PSUM -- compile fails.)
            src = ps[:].rearrange("p g w -> p (g w)")
            dst = ot[:, c % 2].rearrange("p b w -> p (b w)")
            if c % 2 == 0:
                nc.vector.tensor_scalar_max(dst, src, 0.0)
            else:
                nc.scalar.activation(dst, src, relu)
            if c % 2 == 1:
                pair = c // 2
                if pair in GP_PAIRS:
                    q = nc.gpsimd
                else:
                    q = nc.sync if pair % 2 == 0 else nc.scalar
                q.dma_start(y_ap[:, c - 1 : c + 1], ot[:])


def host_weights(a, w1, w2, w3):
    """Fold the 4-way combine into one 9-tap depthwise kernel w_eff."""
    a = np.asarray(a, np.float64)
    w_eff = (
        a[1] * np.asarray(w1, np.float64)[:, 0]
        + a[2] * np.asarray(w2, np.float64)[:, 0]
        + a[3] * np.asarray(w3, np.float64)[:, 0]
    )  # [C, 3, 3]
    w_eff[:, 1, 1] += a[0]
    return w_eff.astype(np.float32)


def host_tridiag(w_eff):
    """[yi, c, dx, yo] tridiag stationary matrices: T[yi,c,dx,yo] =
    w_eff[c, yi-yo+1, dx] for |yi-yo| <= 1."""
    import ml_dtypes

    T = np.zeros((P, C, 3, P), ml_dtypes.bfloat16)
    for dy in range(3):
        yo = np.arange(max(0, 1 - dy), min(P, P + 1 - dy))
        yi = yo + dy - 1
        T[yi, :, :, yo] = w_eff[:, dy, :].astype(ml_dtypes.bfloat16)
    return T


def host_masks_scalars(w_eff):
    """One-hot diagonal masks [y, diag, yo] (diag d hits yo = yi + 1 - d)
    and per-partition-replicated scalars [y, c, dx, diag]."""
    import ml_dtypes

    dmask = np.zeros((P, 3, P), ml_dtypes.bfloat16)
    yi = np.arange(P)
    for d in range(3):
        yo = yi + 1 - d
        v = (yo >= 0) & (yo < P)
        dmask[yi[v], d, yo[v]] = 1.0
    # srep[y, c, dx, d] = w_eff[c, d, dx]
    srep = np.broadcast_to(
        w_eff.transpose(0, 2, 1)[None], (P, C, 3, 3)
    ).astype(np.float32)
    return dmask, np.ascontiguousarray(srep)


def host_inputs(x):
    """[y, c, b, w+2] zero-padded bf16, split per core along c."""
    import ml_dtypes

    xb = np.asarray(x).astype(ml_dtypes.bfloat16)  # [b, c, y, w]
    X = np.zeros((P, C, B, WP), ml_dtypes.bfloat16)
    X[:, :, :, 1 : W + 1] = xb.transpose(2, 1, 0, 3)
    return X


_PROGRAM = None


def _get_program():
    global _PROGRAM
    if _PROGRAM is None:
        nc = bacc.Bacc(
            "TRN2", target_bir_lowering=False, debug=False,
            enable_partition_id=False,
        )
        x_t = nc.dram_tensor("x", [P, CPC, B, WP], BF16, kind="ExternalInput")
        y_t = nc.dram_tensor("y", [P, CPC, B, W], BF16, kind="ExternalOutput")
        kw = {}
        if W_MODE == "host":
            w_t = nc.dram_tensor(
                "w", [P, CPC, 3, P], BF16, kind="ExternalInput"
            )
            args = (y_t.ap(), x_t.ap(), w_t.ap())
        else:
            d_t = nc.dram_tensor("dmask", [P, 3, P], BF16, kind="ExternalInput")
            s_t = nc.dram_tensor(
                "srep", [P, CPC, 3, 3], F32, kind="ExternalInput"
            )
            args = (y_t.ap(), x_t.ap(), None)
            kw = {"dmask_ap": d_t.ap(), "srep_ap": s_t.ap()}
        with TileContext(nc) as tc:
            build_tile_kernel(tc, *args, **kw)
        nc.compile()
        _PROGRAM = nc
    return _PROGRAM


def kernel(x, a, w1, w2, w3, _trace=False, _trace_kwargs=None):
    w_eff = host_weights(a, w1, w2, w3)
    X = host_inputs(x)
    in_maps = []
    if W_MODE == "host":
        T = host_tridiag(w_eff)
        for i in range(NCORES):
            cs = slice(CPC * i, CPC * (i + 1))
            in_maps.append({
                "x": np.ascontiguousarray(X[:, cs]),
                "w": np.ascontiguousarray(T[:, cs]),
            })
    else:
        dmask, srep = host_masks_scalars(w_eff)
        for i in range(NCORES):
            cs = slice(CPC * i, CPC * (i + 1))
            in_maps.append({
                "x": np.ascontiguousarray(X[:, cs]),
                "dmask": dmask,
                "srep": np.ascontiguousarray(srep[:, cs]),
            })
    nc = _get_program()
    res = bass_utils.run_bass_kernel_spmd(
        nc, in_maps, core_ids=list(range(NCORES)), trace=_trace,
        **(_trace_kwargs or {}),
    )
    # res y: [yi, cc, b, w] per core -> out[b, core*CPC+cc, y, w]
    out = np.stack(
        [np.asarray(r["y"], np.float32) for r in res.results], axis=0
    )
    out = out.transpose(3, 0, 2, 1, 4).reshape(B, C, H, W)
    if _trace:
        return out, res
    return out


# revision 30
# speedup vs baseline: 1.0326x; 1.0326x over previous
"""Depthwise 3x3 CNN combo kernel for TRN2 (8 NeuronCores, channel-parallel).

Computes  out = relu(x*a0 + dwconv(x,w1)*a1 + dwconv(x,w2)*a2 + dwconv(x,w3)*a3)
for x [8, 256, 128, 128] f32 by folding everything into a single 9-tap
depthwise conv (conv is linear in the weights; the residual a0*x is the
center tap):  w_eff = a1*w1 + a2*w2 + a3*w3,  w_eff[:,1,1] += a0.

Sharding: CHANNELS across the 8 cores (32 channels x 8 batch images per
core).  Per-core layout puts image ROWS on the partitions:

  x tile  [y=128, (c, b, w=130)]   (w padded 1 left / 1 right with zeros,
                                    host-prepadded so DMA runs are 2080B)

The vertical 3-tap conv then becomes a matmul over the partition (row)
dim with a TRIDIAGONAL stationary matrix T_dx[yi, yo] = w_eff[c, yi-yo+1, dx]
(one matrix per channel and horizontal offset dx).  Each streamed rhs
column picks up all 3 vertical taps at once (384 useful MACs/cycle vs
128 for a diagonal matmul), so the full 9-tap conv needs only 3
accumulating matmuls per psum tile:

  psum[yo, (b, t)] += sum_yi T_dx[yi, yo] * xt[yi, (b, t + dx)]   dx = 0..2

with the dx shift handled as a free-dim offset into the padded row.
Row-boundary zero padding falls out of the band truncating at the matrix
edge.  PSUM accumulates in f32; relu+downcast to bf16 alternates between
DVE and ScalarE (one contiguous 1024-elem op per channel); host upcasts.

The tridiagonal matrices are host-built and DMA'd (3MB; ~10x cheaper
than building them on-chip, where per-partition-varying diagonal writes
are impossible and scalar-broadcast ops run at 1x).  Per-core HBM
traffic: x in 8.5MB, W 3.1MB, y out 8.4MB.
"""

import numpy as np

import concourse.bacc as bacc
import concourse.mybir as mybir
from concourse import bass_utils
from concourse.tile import TileContext

# Problem constants (hardcoded per contract).
B = 8
C = 256
H = 128
W = 128
NCORES = 8

CPC = C // NCORES   # channels per core
P = 128             # partitions (= H rows)
WP = W + 2          # padded row width (1 zero col each side)

F32 = mybir.dt.float32
BF16 = mybir.dt.bfloat16

# "host": DMA full tridiag matrices from HBM.  "dve": build them on-chip
# from diagonal masks + per-channel scalars.  Measured: DVE builds run at
# 1x mode (~1us/matrix, 95us total) while the 3MB host DMA costs ~9us,
# so "host" wins decisively.
W_MODE = "host"


def build_tile_kernel(tc, y_ap, x_ap, w_ap):
    nc = tc.nc
    relu = mybir.ActivationFunctionType.Relu

    with (
        tc.tile_pool(name="xpool", bufs=16) as xpool,
        tc.tile_pool(name="wpool", bufs=8) as wpool,
        tc.tile_pool(name="psum", bufs=4, space="PSUM") as psum_pool,
        tc.tile_pool(name="opool", bufs=4) as opool,
    ):
        # DMA plumbing facts (measured): only sync (SP) and scalar
        # (Activation) have HWDGE queues, ~215GB/s each, with a ~6-deep
        # trigger ring -- a 7th trigger BLOCKS the issuing engine queue.
        # The gpsimd SWDGE path adds ~200GB/s more but starts slowly.
        # Feeding the PE (1.3us/channel) needs ~485GB/s, so: outputs for
        # middle pairs ride SWDGE, x moves in 2-channel chunks and W in
        # 4-channel chunks alternating between the two HWDGE queues, and
        # triggers are emitted just-in-time from inside the channel loop
        # (~8 channels ahead) so ScalarE relus never sit behind a blocked
        # trigger and per-queue FIFO order matches need order.
        GP_PAIRS = (5, 6, 7, 8)  # output pairs on gpsimd SWDGE

        xts = [None] * 16  # 2-channel x chunks, c = 2k
        wts = [None] * 8   # 4-channel W chunks, c = 4j

        def emit_x(k):
            c0 = 2 * k
            xt = xpool.tile([P, 2, B, WP], BF16, name="xt", tag="xt")
            q = nc.sync if k % 2 == 0 else nc.scalar
            q.dma_start(xt[:], x_ap[:, c0 : c0 + 2])
            xts[k] = xt

        def emit_w(j):
            c0 = 4 * j
            wc = wpool.tile([P, 4, 3, P], BF16, name="wc", tag="wc")
            q = nc.scalar if j % 2 == 0 else nc.sync
            q.dma_start(wc[:], w_ap[:, c0 : c0 + 4])
            wts[j] = wc

        # Prefetch channels 0-7 (need order, <=3 triggers per queue).
        emit_x(0); emit_w(0); emit_x(1); emit_x(2); emit_w(1); emit_x(3)

        ot = None
        for c in range(CPC):
            if c % 2 == 0:
                k = c // 2 + 4
                if k < 16:
                    emit_x(k)
            if c % 4 == 0:
                j = c // 4 + 2
                if j < 8:
                    emit_w(j)
            if c % 2 == 0:
                # 2-channel output tile -> one 4KB-run DMA per 2 channels.
                ot = opool.tile([P, 2, B, W], BF16, name="ot", tag="ot")
            # One 2-bank psum tile per channel: 2 groups of 4 images, each
            # matmul exactly one full bank (N = 4*128 = 512, no garbage
            # columns thanks to the per-dx restricted rhs window).  4
            # channels in flight; relu is ONE contiguous op per channel.
            ps = psum_pool.tile([P, 2, 512], F32, name="ps", tag="ps")
            for dx in range(3):
                lhsT = wts[c // 4][:, c % 4, dx, :]
                for g in range(2):
                    nc.tensor.matmul(
                        ps[:, g, :],
                        lhsT=lhsT,
                        rhs=xts[c // 2][:, c % 2, 4 * g : 4 * g + 4,
                                        dx : dx + W],
                        start=(dx == 0),
                        stop=(dx == 2),
                        skip_group_check=True,
                    )
            # relu + bf16 downcast, alternating DVE / ScalarE.  (GpSimd
            # cannot read PSUM -- compile fails.)
            src = ps[:].rearrange("p g w -> p (g w)")
            dst = ot[:, c % 2].rearrange("p b w -> p (b w)")
            if c % 2 == 0:
                nc.vector.tensor_scalar_max(dst, src, 0.0)
            else:
                nc.scalar.activation(dst, src, relu)
            if c % 2 == 1:
                pair = c // 2
                if pair in GP_PAIRS:
                    q = nc.gpsimd
                else:
                    q = nc.sync if pair % 2 == 0 else nc.scalar
                q.dma_start(y_ap[:, c - 1 : c + 1], ot[:])


def host_weights(a, w1, w2, w3):
    """Fold the 4-way combine into one 9-tap depthwise kernel w_eff."""
    a = np.asarray(a, np.float64)
    w_eff = (
        a[1] * np.asarray(w1, np.float64)[:, 0]
        + a[2] * np.asarray(w2, np.float64)[:, 0]
        + a[3] * np.asarray(w3, np.float64)[:, 0]
    )  # [C, 3, 3]
    w_eff[:, 1, 1] += a[0]
    return w_eff.astype(np.float32)


def host_tridiag(w_eff):
    """[yi, c, dx, yo] tridiag stationary matrices: T[yi,c,dx,yo] =
    w_eff[c, yi-yo+1, dx] for |yi-yo| <= 1."""
    import ml_dtypes

    T = np.zeros((P, C, 3, P), ml_dtypes.bfloat16)
    for dy in range(3):
        yo = np.arange(max(0, 1 - dy), min(P, P + 1 - dy))
        yi = yo + dy - 1
        T[yi, :, :, yo] = w_eff[:, dy, :].astype(ml_dtypes.bfloat16)
    return T


def host_masks_scalars(w_eff):
    """One-hot diagonal masks [y, diag, yo] (diag d hits yo = yi + 1 - d)
    and per-partition-replicated scalars [y, c, dx, diag]."""
    import ml_dtypes

    dmask = np.zeros((P, 3, P), ml_dtypes.bfloat16)
    yi = np.arange(P)
    for d in range(3):
        yo = yi + 1 - d
        v = (yo >= 0) & (yo < P)
        dmask[yi[v], d, yo[v]] = 1.0
    # srep[y, c, dx, d] = w_eff[c, d, dx]
    srep = np.broadcast_to(
        w_eff.transpose(0, 2, 1)[None], (P, C, 3, 3)
    ).astype(np.float32)
    return dmask, np.ascontiguousarray(srep)


def host_inputs(x):
    """[y, c, b, w+2] zero-padded bf16, split per core along c."""
    import ml_dtypes

    xb = np.asarray(x).astype(ml_dtypes.bfloat16)  # [b, c, y, w]
    X = np.zeros((P, C, B, WP), ml_dtypes.bfloat16)
    X[:, :, :, 1 : W + 1] = xb.transpose(2, 1, 0, 3)
    return X


_PROGRAM = None


def _get_program():
    global _PROGRAM
    if _PROGRAM is None:
        nc = bacc.Bacc(
            "TRN2", target_bir_lowering=False, debug=False,
            enable_partition_id=False,
        )
        x_t = nc.dram_tensor("x", [P, CPC, B, WP], BF16, kind="ExternalInput")
        y_t = nc.dram_tensor("y", [P, CPC, B, W], BF16, kind="ExternalOutput")
        kw = {}
        if W_MODE == "host":
            w_t = nc.dram_tensor(
                "w", [P, CPC, 3, P], BF16, kind="ExternalInput"
            )
            args = (y_t.ap(), x_t.ap(), w_t.ap())
        else:
            d_t = nc.dram_tensor("dmask", [P, 3, P], BF16, kind="ExternalInput")
            s_t = nc.dram_tensor(
                "srep", [P, CPC, 3, 3], F32, kind="ExternalInput"
            )
            args = (y_t.ap(), x_t.ap(), None)
            kw = {"dmask_ap": d_t.ap(), "srep_ap": s_t.ap()}
        with TileContext(nc) as tc:
            build_tile_kernel(tc, *args, **kw)
        nc.compile()
        _PROGRAM = nc
    return _PROGRAM


def kernel(x, a, w1, w2, w3, _trace=False, _trace_kwargs=None):
    w_eff = host_weights(a, w1, w2, w3)
    X = host_inputs(x)
    in_maps = []
    if W_MODE == "host":
        T = host_tridiag(w_eff)
        for i in range(NCORES):
            cs = slice(CPC * i, CPC * (i + 1))
            in_maps.append({
                "x": np.ascontiguousarray(X[:, cs]),
                "w": np.ascontiguousarray(T[:, cs]),
            })
    else:
        dmask, srep = host_masks_scalars(w_eff)
        for i in range(NCORES):
            cs = slice(CPC * i, CPC * (i + 1))
            in_maps.append({
                "x": np.ascontiguousarray(X[:, cs]),
                "dmask": dmask,
                "srep": np.ascontiguousarray(srep[:, cs]),
            })
    nc = _get_program()
    res = bass_utils.run_bass_kernel_spmd(
        nc, in_maps, core_ids=list(range(NCORES)), trace=_trace,
        **(_trace_kwargs or {}),
    )
    # res y: [yi, cc, b, w] per core -> out[b, core*CPC+cc, y, w]
    out = np.stack(
        [np.asarray(r["y"], np.float32) for r in res.results], axis=0
    )
    out = out.transpose(3, 0, 2, 1, 4).reshape(B, C, H, W)
    if _trace:
        return out, res
    return out
